# revision 35
# baseline (speedup 1.0000x reference)
"""Trainium2 Bass kernel v2 for nn_Block_85126251807269 (RetNet-style block).

Sharding: token-parallel over 8 NeuronCores (core c -> batch c//4, tokens
[1024*(c%4), 1024*(c%4+1))). Weights replicated (bf16). Cross-core comm is
one AllGather of per-head retention segment states (512KB/core) per half.

Key differences vs v1:
  - bf16 matmul operands everywhere (fp32 PSUM accumulation): 1 cyc/row
    even for the small N=128 retention matmuls (fp32 was 4 cyc/row), and
    half the weight DMA traffic.
  - Retention state chain reformulated: with kdk'_i = k_i*b^(C-1-s)*dc^-(i+1)
    and qdq'_i = q_i*b^(t+1)/sqrt(dk) (t global in segment), the state is a
    pure sum T_i = sum_j K_j (K_j = kdk'_j^T v_j), so there is no per-chunk
    scale+add serial chain; per-chunk state snapshots are single DVE adds.
  - x2 and qdq kept in SBUF (no DRAM roundtrips); ~60 large DMAs instead of
    ~780 small ones.
  - FFN: y accumulated in SBUF fp32 across 4 kt-groups; PSUM banks released
    per (tt,n) so fc1/fc2 pipeline without bank deadlocks.
  - Cross-core correction o += qdq' @ S_init applied directly to obuf
    (qdq' already carries the dc^i chunk scaling, so no rescale needed).
"""
import sys

sys.path.insert(0, "/opt/trn_rl_repo")
import antenv  # noqa: E402

if not hasattr(antenv, "axon_hooks"):
    import importlib.util

    _spec = importlib.util.spec_from_file_location(
        "antenv.axon_hooks", "/opt/trn_rl_repo/antenv/axon_hooks.py")
    if _spec is not None and _spec.loader is not None:
        try:
            _m = importlib.util.module_from_spec(_spec)
            _spec.loader.exec_module(_m)
            sys.modules["antenv.axon_hooks"] = _m
            antenv.axon_hooks = _m
        except Exception:
            pass

import numpy as np  # noqa: E402
import ml_dtypes  # noqa: E402
import concourse.bacc as bacc  # noqa: E402
import concourse.mybir as mybir  # noqa: E402
import concourse.tile as tile  # noqa: E402
from concourse.bass_utils import run_bass_kernel_spmd  # noqa: E402
from concourse.masks import make_identity  # noqa: E402
from concourse.tile_rust import add_dep_helper  # noqa: E402

dt = mybir.dt
AF = mybir.ActivationFunctionType
ALU = mybir.AluOpType
PM = mybir.MatmulPerfMode
BF = dt.bfloat16
F32 = dt.float32
F8 = dt.float8e4
BF_NP = ml_dtypes.bfloat16
F8_NP = ml_dtypes.float8_e4m3

B, L, D, H = 2, 4096, 1024, 8
DK, DV, FFN = 128, 256, 4096
NCORES = 8
SEG = 1024          # tokens per core
PT = SEG // 128     # token tiles per core
C = 128             # retention chunk
NCH = SEG // C      # chunks per core
KO = D // 128       # contraction tiles over D
FT = FFN // 128     # ffn col tiles
EPS = 1e-6

_b64 = (1.0 - np.exp2(-5.0 - np.arange(H))).astype(np.float64)
_logb = np.log(_b64)

_NC_CACHE = {}


def _build():
    nc = bacc.Bacc("TRN2", target_bir_lowering=False, debug=False,
                   num_devices=NCORES)

    def din(name, shape, d=F32):
        return nc.dram_tensor(name, list(shape), d, kind="ExternalInput")

    x_d = din("x", [SEG, D], BF)
    wqkv_d = din("wqkv", [4, 128, KO, 1024], BF)  # per head-pair:
    # cols = [q_a(128) | k_a(128) | q_b(128) | k_b(128) | v_a(256) | v_b(256)]
    wo_d = din("wo2", [2, 128, 2, 8, 512], BF)    # [half,p,n,r,c]; r=hh*2+j
    w1_d = din("w1t", [4, 128, 8, KO, 128], F8)   # [g,p,kt,ko,m] (x256)
    w2_d = din("w2t", [4, 128, 8, 1024], F8)      # [g,p,kt,c]
    maskT_d = din("maskT2", [128, H, 512], BF)    # [s,h,(4,t)] decayed mask^T
    dqb_d = din("dqbf", [128, H, C], BF)          # [p,h,tau] = b^(tau+1)
    dkcol_d = din("dkcol2", [128, H, NCH])        # [s,h,i]=b^(127-s)*dc^-(i+1)
    coef_d = din("coefT", [128, H, 4])            # per-core prefix coefs
    b1T_d = din("b1T", [128, FT])

    y_d = nc.dram_tensor("y", [SEG, D], BF, kind="ExternalOutput")
    yr = y_d.ap().rearrange("(tt p) d -> tt p d", p=128)
    xr = x_d.ap().rearrange("(tt p) d -> tt p d", p=128)

    dc8 = [float(_b64[h] ** SEG) for h in range(H)]   # dc^NCH = b^1024
    dci = [[float(_b64[h] ** (C * i)) for i in range(NCH)] for h in range(H)]

    with tile.TileContext(nc) as tc:
        with tc.tile_pool(name="persist", bufs=1) as P, \
             tc.tile_pool(name="tabs", bufs=1) as TB, \
             tc.tile_pool(name="stats", bufs=4) as SP, \
             tc.tile_pool(name="htp", bufs=2) as HTP, \
             tc.tile_pool(name="ps", bufs=6, space="PSUM") as PS, \
             tc.tile_pool(name="ps2", bufs=2, space="PSUM") as PS2, \
             tc.tile_pool(name="dram", bufs=1, space="DRAM") as DR:

            def ps(n=512):
                return PS.tile([128, 512], F32, tag="ps", name="ps")[:, :n]

            def psb(n=512):
                return PS.tile([128, 512], BF, tag="ps", name="psb")[:, :n]

            def ps2(n=512):
                return PS2.tile([128, 512], F32, tag="po", name="po")[:, :n]

            # ---- persistent SBUF ----
            x_sb = P.tile([128, PT, D], BF, name="x_sb")        # x then x2
            hT = P.tile([128, KO, SEG], BF, name="hT")          # h^T (phase 1)

            # ---- DRAM scratch for AllGather (2 groups: h4-7 first, h0-3) --
            _aghs = [(4, 4), (0, 4)]   # (first head, count) per AG group
            cin_h = [DR.tile([128, 4, DV], BF, name=f"cih{i}")
                     for i in range(2)]
            cout_h = [DR.tile([512, 4, DV], BF, name=f"coh{i}")
                      for i in range(2)]

            # ---- tables ----
            dqb = TB.tile([128, H, C], BF, name="dqb")
            nc.sync.dma_start(dqb[:], dqb_d.ap())
            dkcol = TB.tile([128, H, NCH], F32, name="dkcol")
            nc.sync.dma_start(dkcol[:], dkcol_d.ap())
            coefT = TB.tile([128, H, 4], F32, name="coefT")
            nc.sync.dma_start(coefT[:], coef_d.ap())
            b1T = TB.tile([128, FT], F32, name="b1T")
            nc.sync.dma_start(b1T[:], b1T_d.ap())
            eps_t = TB.tile([128, 1], F32, name="eps")
            nc.vector.memset(eps_t[:], EPS)
            eps2_t = TB.tile([128, 1], F32, name="eps2")
            nc.vector.memset(eps2_t[:], EPS / 256.0)
            identb = TB.tile([128, 128], BF, name="identb")
            make_identity(nc, identb[:])

            ns = nc.named_scope

            # ============ LayerNorm + transpose into hT / h8 ============
            # Phase 2 emits 16*h2 in fp8e4 (scale folded into rstd via the
            # sqrt((var+eps)/256) trick); fc1 descales via gelu scale=2^-12.
            def ln_phase(tag, dst_t):
                ph2 = tag == "2"
                for tt in range(PT):
                  with ns(f"ln{tag}_{tt}"):
                    if tag == "1" and tt > 0:   # tt0 DMA hoisted (startup)
                        nc.sync.dma_start(x_sb[:, tt, :], xr[tt])
                    st = SP.tile([128, 2, 6], F32, tag="bst")
                    xg = x_sb[:, tt, :].rearrange("p (s f) -> p s f", f=512)
                    for s in range(2):
                        nc.vector.bn_stats(out=st[:, s, :], in_=xg[:, s, :])
                    mv = SP.tile([128, 2], F32, tag="mv")
                    nc.vector.bn_aggr(out=mv[:], in_=st[:])
                    std = SP.tile([128, 1], F32, tag="std")
                    nc.scalar.activation(std[:], mv[:, 1:2], AF.Sqrt,
                                         bias=eps2_t[:] if ph2 else eps_t[:],
                                         scale=1.0 / 256.0 if ph2 else 1.0)
                    rstd = SP.tile([128, 1], F32, tag="rstd")
                    nc.vector.reciprocal(rstd[:], std[:])
                    nb = SP.tile([128, 1], F32, tag="nb")
                    nc.vector.tensor_scalar(
                        out=nb[:], in0=rstd[:],
                        scalar1=mv[:, 0:1], scalar2=-1.0,
                        op0=ALU.mult, op1=ALU.mult)
                    htmp = HTP.tile([128, D], BF, tag="htmp")
                    nc.scalar.activation(htmp[:], x_sb[:, tt, :], AF.Identity,
                                         bias=nb[:], scale=rstd[:])
                    for g in range(2):
                        pt_ = psb()
                        for k4 in range(4):
                            ko = g * 4 + k4
                            nc.tensor.transpose(
                                pt_[:, k4 * 128:(k4 + 1) * 128],
                                htmp[:, ko * 128:(ko + 1) * 128], identb[:])
                        dst = dst_t[:, g * 4:(g + 1) * 4,
                                    tt * 128:(tt + 1) * 128]
                        src = pt_[:].rearrange("p (k f) -> p k f", f=128)
                        if g == 0:   # split copies across engines: the LN
                            nc.scalar.copy(dst, src)   # chain is DVE-bound
                        else:
                            nc.vector.tensor_copy(out=dst, in_=src)

            # ============ per-head retention ============
            with tc.tile_pool(name="retp", bufs=1) as RP, \
                 tc.tile_pool(name="sgp", bufs=1) as SGP, \
                 tc.tile_pool(name="ret", bufs=2) as RET:

              qdqT = RP.tile([128, H, SEG], BF, name="qdqT")    # q*b^(t+1)/sq
              obuf = RP.tile([128, 2, 8, SEG], BF, name="obuf")  # o^T halves
              S = RP.tile([128, H, DV], BF, name="S")           # seg state
              Sb = RP.tile([128, H, DV], BF, name="Sb")         # S_init bf16
              maskT = RP.tile([128, H, 512], BF, name="maskT")

              # startup order: x tile 0 (longest LN chain) first, then the
              # first weight pair (so QKV can start asap), then tables.
              nc.sync.dma_start(x_sb[:, 0, :], xr[0])
              wp2 = RET.tile([128, KO, 1024], BF, tag="wqkv", name="wqkv")
              nc.sync.dma_start(wp2[:], wqkv_d.ap()[2])
              nc.sync.dma_start(maskT[:], maskT_d.ap())

              ln_phase("1", hT)

              _anchor = [None, None]   # last retention [PE matmul, DVE copy]

              def qk_head(h, wp, qT, kT):
                hh2 = h % 2
                # stationary weight reused for both token halves
                for w0, dstT in ((hh2 * 256, qT), (hh2 * 256 + 128, kT)):
                    pp = [ps(), ps()]
                    for ko in range(KO):
                        for n in range(2):
                            nc.tensor.matmul(
                                pp[n][:], wp[:, ko, w0:w0 + 128],
                                hT[:, ko, n * 512:(n + 1) * 512],
                                start=(ko == 0), stop=(ko == KO - 1))
                    for n in range(2):
                        nc.scalar.copy(dstT[:, n * 512:(n + 1) * 512],
                                       pp[n][:])
                # qdq'_i = q_i * b^(tau+1) * dc^i  (Wq carries 1/sqrt(dk))
                for i in range(NCH):
                    isl = slice(i * 128, (i + 1) * 128)
                    nc.vector.tensor_tensor(out=qdqT[:, h, isl],
                                            in0=qT[:, isl],
                                            in1=dqb[:, h, :], op=ALU.mult)
                    if i > 0:
                        nc.vector.tensor_scalar_mul(
                            out=qdqT[:, h, isl], in0=qdqT[:, h, isl],
                            scalar1=dci[h][i])

              def ret_head(h, qT, kT, vn, vof):
                half, hh = h // 4, h % 4
                # kdk' = k^T-chunk * b^(127-s) * dc^-(i+1)
                kdk = RET.tile([128, NCH, 128], BF, tag="kdk", name="kdk")
                for ig in range(2):
                    pt_ = psb()
                    for k4 in range(4):
                        i = ig * 4 + k4
                        isl = slice(i * 128, (i + 1) * 128)
                        nc.tensor.transpose(pt_[:, k4 * 128:(k4 + 1) * 128],
                                            kT[:, isl], identb[:])
                    for k4 in range(4):
                        i = ig * 4 + k4
                        nc.scalar.activation(
                            kdk[:, i, :], pt_[:, k4 * 128:(k4 + 1) * 128],
                            AF.Copy, scale=dkcol[:, h, i:i + 1])
                # state snapshots T_i = sum_{j<=i} kdk'_j^T v_j -- FIRST, so
                # the segment-final state (feeding the AllGather) is ready
                # before the o-computation fills the PE.
                Ts = RET.tile([128, NCH, DV], BF, tag="Ts", name="Ts")
                for i in range(NCH):
                    px = ps(256)
                    nc.tensor.matmul(px[:], kdk[:, i, :],
                                     vn[:, i, vof:vof + 256],
                                     start=True, stop=True)
                    if i == 0:
                        nc.vector.tensor_copy(out=Ts[:, 0, :], in_=px[:])
                    else:
                        nc.vector.tensor_tensor(out=Ts[:, i, :], in0=px[:],
                                                in1=Ts[:, i - 1, :],
                                                op=ALU.add)
                # segment-final state for AllGather: S = dc^NCH * T_7
                nc.scalar.activation(S[:, h, :], Ts[:, NCH - 1, :],
                                     AF.Copy, scale=dc8[h])
                # A^T quads + mask
                At = RET.tile([128, NCH, 128], BF, tag="At", name="At")
                for ip in range(2):
                    pa = ps()
                    for i4 in range(4):
                        i = ip * 4 + i4
                        isl = slice(i * 128, (i + 1) * 128)
                        nc.tensor.matmul(pa[:, i4 * 128:(i4 + 1) * 128],
                                         kT[:, isl], qT[:, isl],
                                         start=True, stop=True)
                    nc.vector.tensor_tensor(
                        out=At[:, ip * 4:(ip + 1) * 4, :].rearrange(
                            "p k f -> p (k f)"),
                        in0=pa[:], in1=maskT[:, h, :], op=ALU.mult)
                # o^T = (A*mask)^T v + qdq' @ T_(i-1), chunk pairs
                for ip in range(4):
                    po = ps2()
                    for i2 in range(2):
                        i = ip * 2 + i2
                        isl = slice(i * 128, (i + 1) * 128)
                        for j in range(2):
                            col = j * 256 + i2 * 128
                            jsl = slice(vof + j * 128, vof + (j + 1) * 128)
                            mm = nc.tensor.matmul(po[:, col:col + 128],
                                                  vn[:, i, jsl], At[:, i, :],
                                                  start=True, stop=(i == 0))
                            if i > 0:
                                mm = nc.tensor.matmul(
                                    po[:, col:col + 128],
                                    Ts[:, i - 1, j * 128:(j + 1) * 128],
                                    qdqT[:, h, isl],
                                    start=False, stop=True)
                            _anchor[0] = mm
                    dst = obuf[:, half, hh * 2:hh * 2 + 2,
                               ip * 256:(ip + 1) * 256]
                    _anchor[1] = nc.vector.tensor_copy(
                        out=dst,
                        in_=po[:].rearrange("p (j f) -> p j f", f=256))

              def calc_vn(a, wp):
                # v for both heads: [s, v_a(256)|v_b(256)]
                with ns(f"head{a}"):
                    vn = RET.tile([128, NCH, 512], BF, tag="vn", name="vn")
                    for i in range(NCH):
                        isl = slice(i * 128, (i + 1) * 128)
                        pv = ps()
                        for ko in range(KO):
                            nc.tensor.matmul(pv[:], hT[:, ko, isl],
                                             wp[:, ko, 512:1024],
                                             start=(ko == 0),
                                             stop=(ko == KO - 1))
                        nc.scalar.copy(vn[:, i, :], pv[:])
                return vn

              def pair(p, wp=None, v_first=False):
                a, b = 2 * p, 2 * p + 1
                # v_first: v chunk i only needs hT tile i, so at kernel start
                # the PE can begin before the whole LN phase has finished
                # (the paired q/k matmuls touch all of hT at once).
                if v_first:
                    vn = calc_vn(a, wp)
                with ns(f"head{a}"):
                    if wp is None:
                        wp = RET.tile([128, KO, 1024], BF, tag="wqkv",
                                      name="wqkv")
                        nc.sync.dma_start(wp[:], wqkv_d.ap()[p])
                    qTa = RET.tile([128, SEG], BF, tag="qT", name="qT")
                    kTa = RET.tile([128, SEG], BF, tag="kT", name="kT")
                    qk_head(a, wp, qTa, kTa)
                with ns(f"head{b}"):
                    qTb = RET.tile([128, SEG], BF, tag="qT", name="qT")
                    kTb = RET.tile([128, SEG], BF, tag="kT", name="kT")
                    qk_head(b, wp, qTb, kTb)
                if not v_first:
                    vn = calc_vn(a, wp)
                with ns(f"head{a}"):
                    ret_head(a, qTa, kTa, vn, 0)
                with ns(f"head{b}"):
                    ret_head(b, qTb, kTb, vn, 256)

              _sgg = {}

              def ag(gi):
                h0, cnt = _aghs[gi]
                with ns(f"ag{gi}"):
                    nc.gpsimd.dma_start(cin_h[gi][:], S[:, h0:h0 + cnt, :])
                    nc.gpsimd.collective_compute(
                        "AllGather", ALU.bypass,
                        replica_groups=[[0, 1, 2, 3], [4, 5, 6, 7]],
                        ins=[cin_h[gi].opt()], outs=[cout_h[gi].opt()])
                    # gather the whole group's states in ONE DMA (a tile per
                    # group: no buffer aliasing, no WAR chain on the queue)
                    coutv = cout_h[gi][:].rearrange(
                        "(j p) h v -> p j h v", p=128)
                    Sgg = SGP.tile([128, 4, 4, DV], BF, tag="Sgg", name="Sgg")
                    nc.gpsimd.dma_start(Sgg[:], coutv)
                    _sgg[gi] = Sgg

              # halves 4-7 first: their AG overlaps heads 0-3; the final AG
              # (heads 0-3) hides under corr4-7 + wo1.
              pair(2, wp2, v_first=True)
              pair(3)
              ag(0)
              pair(0)
              pair(1)
              ag(1)

              # ---- S_init per head (post-AG) on GpSimd DMA + DVE math,
              # issued after all retention DVE work and pinned behind it
              # (the in-order DVE queue would otherwise stall retention
              # behind the collective-gated ops) ----
              def sinit(h):
                gi = 0 if h >= 4 else 1
                h0, cnt = _aghs[gi]
                with ns(f"sinit{h}"):
                    Sg = _sgg[gi]
                    Si = Sb[:, h, :]
                    op0 = nc.vector.tensor_scalar_mul(
                        out=Si, in0=Sg[:, 0, h - h0, :],
                        scalar1=coefT[:, h, 0:1])
                    if _anchor[1] is not None:
                        add_dep_helper(op0.ins, _anchor[1].ins, sync=False,
                                       reason="sinit after retention DVE")
                    for j in range(1, 4):
                        nc.vector.scalar_tensor_tensor(
                            out=Si, in0=Sg[:, j, h - h0, :],
                            scalar=coefT[:, h, j:j + 1], in1=Si,
                            op0=ALU.mult, op1=ALU.add)

              # ---- correction: obuf += (qdq' @ S_init)^T ----
              def corr(h):
                half, hh = h // 4, h % 4
                with ns(f"corr{h}"):
                    for n in range(2):
                        nsl = slice(n * 512, (n + 1) * 512)
                        for j in range(2):
                            pc = ps()
                            mm = nc.tensor.matmul(
                                pc[:], Sb[:, h, j * 128:(j + 1) * 128],
                                qdqT[:, h, nsl], start=True, stop=True)
                            if _anchor[0] is not None:
                                add_dep_helper(mm.ins, _anchor[0].ins,
                                               sync=False,
                                               reason="corr after retention")
                            dst = obuf[:, half, hh * 2 + j, nsl]
                            nc.vector.tensor_tensor(out=dst, in0=pc[:],
                                                    in1=dst, op=ALU.add)

              # ---- Wo ----
              def wo_round(half):
                wons = []
                for n in range(2):
                    won = RET.tile([128, 8, 512], BF, tag="won", name="won")
                    nc.sync.dma_start(won[:], wo_d.ap()[half, :, n])
                    wons.append(won)
                for tt in range(PT):
                    tsl = slice(tt * 128, (tt + 1) * 128)
                    with ns(f"wo{half}_{tt}"):
                        pw = [ps(), ps()]
                        for r in range(8):     # stationary reused for both n
                            for n in range(2):
                                nc.tensor.matmul(
                                    pw[n][:], obuf[:, half, r, tsl],
                                    wons[n][:, r, :],
                                    start=(r == 0), stop=(r == 7))
                        for n in range(2):
                            nsl = slice(n * 512, (n + 1) * 512)
                            nc.vector.tensor_tensor(out=x_sb[:, tt, nsl],
                                                    in0=pw[n][:],
                                                    in1=x_sb[:, tt, nsl],
                                                    op=ALU.add)

              for h in range(4, 8):
                  sinit(h)
                  corr(h)
              wo_round(1)
              for h in range(4):
                  sinit(h)
                  corr(h)
              wo_round(0)

            # ============ LN2 + FFN (x_sb now holds x2) ============
            with tc.tile_pool(name="ffn", bufs=2) as FP, \
                 tc.tile_pool(name="yap", bufs=1) as YA:
                h8 = YA.tile([128, KO, SEG], F8, name="h8")     # 16*h2^T fp8
                ln_phase("2", h8)
                y_acc = YA.tile([128, PT, D], F32, name="yacc")
                for g in range(4):
                    w1g = FP.tile([128, 8, KO, 128], F8, tag="w1g",
                                  name="w1g")
                    nc.sync.dma_start(w1g[:], w1_d.ap()[g])
                    w2g = FP.tile([128, 8, 1024], F8, tag="w2g", name="w2g")
                    nc.sync.dma_start(w2g[:], w2_d.ap()[g])
                    fT = FP.tile([128, 8, 2, 512], F8, tag="fT", name="fT")
                    for kt in range(8):
                      with ns(f"ffn{g}_f{kt}"):
                        ktr = g * 8 + kt
                        pf = [ps(), ps()]
                        for k2 in range(KO // 2):
                            k2s = slice(2 * k2, 2 * k2 + 2)
                            for th in range(2):  # stationary reused
                                nc.tensor.matmul(
                                    pf[th][:], w1g[:, kt, k2s, :],
                                    h8[:, k2s, th * 512:(th + 1) * 512],
                                    start=(k2 == 0),
                                    stop=(k2 == KO // 2 - 1),
                                    perf_mode=PM.DoubleRow)
                        for th in range(2):
                            nc.scalar.activation(fT[:, kt, th, :], pf[th][:],
                                                 AF.Gelu,
                                                 bias=b1T[:, ktr:ktr + 1],
                                                 scale=1.0 / 4096.0)
                    for th in range(2):
                        for tt4 in range(4):
                            tt = th * 4 + tt4
                            t4 = slice(tt4 * 128, (tt4 + 1) * 128)
                            for n in range(2):
                              with ns(f"ffn{g}_{th}_y{tt4}_{n}"):
                                nsl = slice(n * 512, (n + 1) * 512)
                                pg = ps2()
                                for k2 in range(4):
                                    k2s = slice(2 * k2, 2 * k2 + 2)
                                    nc.tensor.matmul(
                                        pg[:], fT[:, k2s, th, t4],
                                        w2g[:, k2s, nsl],
                                        start=(k2 == 0), stop=(k2 == 3),
                                        perf_mode=PM.DoubleRow)
                                if g == 0:
                                    nc.vector.tensor_tensor(
                                        out=y_acc[:, tt, nsl], in0=pg[:],
                                        in1=x_sb[:, tt, nsl], op=ALU.add)
                                elif g < 3:
                                    nc.vector.tensor_tensor(
                                        out=y_acc[:, tt, nsl], in0=pg[:],
                                        in1=y_acc[:, tt, nsl], op=ALU.add)
                                else:
                                    yt = FP.tile([128, 512], BF, tag="yt",
                                                 name="yt")
                                    nc.vector.tensor_tensor(
                                        out=yt[:], in0=pg[:],
                                        in1=y_acc[:, tt, nsl], op=ALU.add)
                                    nc.sync.dma_start(yr[tt][:, nsl], yt[:])

    nc.compile()
    return nc


def _host_prep(inputs):
    x = np.asarray(inputs["x"], np.float32)
    ln1_w = np.asarray(inputs["ln1_w"], np.float32)
    ln1_b = np.asarray(inputs["ln1_b"], np.float32)
    Wq = np.asarray(inputs["Wq"], np.float32)
    Wk = np.asarray(inputs["Wk"], np.float32)
    Wv = np.asarray(inputs["Wv"], np.float32)
    Wo = np.asarray(inputs["Wo"], np.float32)
    ln2_w = np.asarray(inputs["ln2_w"], np.float32)
    ln2_b = np.asarray(inputs["ln2_b"], np.float32)
    W1 = np.asarray(inputs["W1"], np.float32)
    b1 = np.asarray(inputs["b1"], np.float32)
    W2 = np.asarray(inputs["W2"], np.float32)
    b2 = np.asarray(inputs["b2"], np.float32)

    assert np.all(ln1_b == 0) and np.all(ln2_b == 0) and np.all(b2 == 0), \
        "kernel build assumes zero ln1_b/ln2_b/b2 (gated paths not emitted)"

    sc = 1.0 / np.sqrt(np.float64(DK))
    wq_e = ln1_w[:, None] * Wq * sc     # fold 1/sqrt(dk) into Wq
    wk_e = ln1_w[:, None] * Wk
    wv_e = ln1_w[:, None] * Wv
    w1_e = ln2_w[:, None] * W1

    def bf(a):
        return np.ascontiguousarray(a).astype(BF_NP)

    def f8(a):
        return np.ascontiguousarray(np.clip(a, -240.0, 240.0)).astype(F8_NP)

    wqh = wq_e.reshape(KO, 128, H, 128).transpose(2, 1, 0, 3)
    wkh = wk_e.reshape(KO, 128, H, 128).transpose(2, 1, 0, 3)
    wvh = wv_e.reshape(KO, 128, H, 256).transpose(2, 1, 0, 3)
    # per head-pair: [qa | ka | qb | kb | va | vb] -> [4, 128, KO, 1024]
    wqkv = bf(np.concatenate(
        [np.stack([wqh[0::2], wkh[0::2], wqh[1::2], wkh[1::2]], axis=0)
         .transpose(1, 2, 3, 0, 4).reshape(4, 128, KO, 512),
         np.concatenate([wvh[0::2], wvh[1::2]], axis=-1)], axis=-1))

    wo2 = bf(Wo.reshape(2, 8, 128, 2, 512).transpose(0, 2, 3, 1, 4))
    w1t = f8((w1_e * 256.0).reshape(KO, 128, FT, 128).transpose(2, 1, 0, 3)
             .reshape(4, 8, 128, KO, 128).transpose(0, 2, 1, 3, 4))
    w2t = f8(W2.reshape(4, 8, 128, 1024).transpose(0, 2, 1, 3))

    t_ = np.arange(C, dtype=np.float64)
    maskT2 = np.zeros((128, H, 512), np.float64)
    dqbf = np.zeros((128, H, C), np.float64)
    dkcol2 = np.zeros((128, H, NCH), np.float32)
    for h in range(H):
        diff = t_[None, :] - t_[:, None]      # [s, t] -> t - s
        m = np.where(diff >= 0, np.exp(_logb[h] * diff), 0.0)
        for r4 in range(4):
            maskT2[:, h, r4 * 128:(r4 + 1) * 128] = m
        dqbf[:, h, :] = np.exp(_logb[h] * (t_ + 1.0))[None, :]
        dc = np.exp(_logb[h] * C)
        for i in range(NCH):
            dkcol2[:, h, i] = (np.exp(_logb[h] * (C - 1.0 - t_))
                               * dc ** (-(i + 1.0)))
    maskT2 = bf(maskT2)
    dqbf = bf(dqbf)

    b1_e = b1 + ln2_b @ W1
    b1T = np.ascontiguousarray(b1_e.reshape(FT, 128).T).astype(np.float32)

    shared = dict(wqkv=wqkv, wo2=wo2, w1t=w1t, w2t=w2t,
                  maskT2=maskT2, dqbf=dqbf, dkcol2=dkcol2, b1T=b1T)

    in_maps = []
    for c in range(NCORES):
        b, s = c // 4, c % 4
        coefT = np.zeros((128, H, 4), np.float32)
        for h in range(H):
            for j in range(4):
                if j < s:
                    coefT[:, h, j] = np.exp(_logb[h] * (SEG * (s - 1 - j)))
        m = dict(shared)
        m["x"] = bf(x[b, s * SEG:(s + 1) * SEG, :])
        m["coefT"] = coefT
        in_maps.append(m)
    return in_maps


def kernel(**inputs):
    if "nc" not in _NC_CACHE:
        _NC_CACHE["nc"] = _build()
    nc = _NC_CACHE["nc"]
    in_maps = _host_prep(inputs)
    res = run_bass_kernel_spmd(nc, in_maps, core_ids=list(range(NCORES)))
    _NC_CACHE["last_res"] = res
    out = np.zeros((B, L, D), np.float32)
    for c in range(NCORES):
        b, s = c // 4, c % 4
        out[b, s * SEG:(s + 1) * SEG, :] = res.results[c]["y"]
    return out



# revision 40
# speedup vs baseline: 1.0192x; 1.0192x over previous
"""Trainium2 Bass kernel v2 for nn_Block_85126251807269 (RetNet-style block).

Sharding: token-parallel over 8 NeuronCores (core c -> batch c//4, tokens
[1024*(c%4), 1024*(c%4+1))). Weights replicated (bf16). Cross-core comm is
one AllGather of per-head retention segment states (512KB/core) per half.

Key differences vs v1:
  - bf16 matmul operands everywhere (fp32 PSUM accumulation): 1 cyc/row
    even for the small N=128 retention matmuls (fp32 was 4 cyc/row), and
    half the weight DMA traffic.
  - Retention state chain reformulated: with kdk'_i = k_i*b^(C-1-s)*dc^-(i+1)
    and qdq'_i = q_i*b^(t+1)/sqrt(dk) (t global in segment), the state is a
    pure sum T_i = sum_j K_j (K_j = kdk'_j^T v_j), so there is no per-chunk
    scale+add serial chain; per-chunk state snapshots are single DVE adds.
  - x2 and qdq kept in SBUF (no DRAM roundtrips); ~60 large DMAs instead of
    ~780 small ones.
  - FFN: y accumulated in SBUF fp32 across 4 kt-groups; PSUM banks released
    per (tt,n) so fc1/fc2 pipeline without bank deadlocks.
  - Cross-core correction o += qdq' @ S_init applied directly to obuf
    (qdq' already carries the dc^i chunk scaling, so no rescale needed).
"""
import sys

sys.path.insert(0, "/opt/trn_rl_repo")
import antenv  # noqa: E402

if not hasattr(antenv, "axon_hooks"):
    import importlib.util

    _spec = importlib.util.spec_from_file_location(
        "antenv.axon_hooks", "/opt/trn_rl_repo/antenv/axon_hooks.py")
    if _spec is not None and _spec.loader is not None:
        try:
            _m = importlib.util.module_from_spec(_spec)
            _spec.loader.exec_module(_m)
            sys.modules["antenv.axon_hooks"] = _m
            antenv.axon_hooks = _m
        except Exception:
            pass

import numpy as np  # noqa: E402
import ml_dtypes  # noqa: E402
import concourse.bacc as bacc  # noqa: E402
import concourse.mybir as mybir  # noqa: E402
import concourse.tile as tile  # noqa: E402
from concourse.bass_utils import run_bass_kernel_spmd  # noqa: E402
from concourse.masks import make_identity  # noqa: E402
from concourse.tile_rust import add_dep_helper  # noqa: E402

dt = mybir.dt
AF = mybir.ActivationFunctionType
ALU = mybir.AluOpType
PM = mybir.MatmulPerfMode
BF = dt.bfloat16
F32 = dt.float32
F8 = dt.float8e4
BF_NP = ml_dtypes.bfloat16
F8_NP = ml_dtypes.float8_e4m3

B, L, D, H = 2, 4096, 1024, 8
DK, DV, FFN = 128, 256, 4096
NCORES = 8
SEG = 1024          # tokens per core
PT = SEG // 128     # token tiles per core
C = 128             # retention chunk
NCH = SEG // C      # chunks per core
KO = D // 128       # contraction tiles over D
FT = FFN // 128     # ffn col tiles
EPS = 1e-6

_b64 = (1.0 - np.exp2(-5.0 - np.arange(H))).astype(np.float64)
_logb = np.log(_b64)

_NC_CACHE = {}


def _build():
    nc = bacc.Bacc("TRN2", target_bir_lowering=False, debug=False,
                   num_devices=NCORES)

    def din(name, shape, d=F32):
        return nc.dram_tensor(name, list(shape), d, kind="ExternalInput")

    x_d = din("x", [SEG, D], BF)
    wqkv_d = din("wqkv", [4, 128, KO, 1024], BF)  # per head-pair:
    # cols = [q_a(128) | k_a(128) | q_b(128) | k_b(128) | v_a(256) | v_b(256)]
    wo_d = din("wo2", [2, 128, 2, 8, 512], BF)    # [half,p,n,r,c]; r=hh*2+j
    w1_d = din("w1t", [4, 128, 8, KO, 128], F8)   # [g,p,kt,ko,m] (x256)
    w2_d = din("w2t", [4, 128, 8, 1024], F8)      # [g,p,kt,c]
    maskT_d = din("maskT2", [128, H, 512], BF)    # [s,h,(4,t)] decayed mask^T
    dqb_d = din("dqbf", [128, H, C], BF)          # [p,h,tau] = b^(tau+1)
    dkcol_d = din("dkcol2", [128, H, NCH])        # [s,h,i]=b^(127-s)*dc^-(i+1)
    coef_d = din("coefT", [128, H, 4])            # per-core prefix coefs
    b1T_d = din("b1T", [128, FT])

    y_d = nc.dram_tensor("y", [SEG, D], BF, kind="ExternalOutput")
    yr = y_d.ap().rearrange("(tt p) d -> tt p d", p=128)
    xr = x_d.ap().rearrange("(tt p) d -> tt p d", p=128)

    dc8 = [float(_b64[h] ** SEG) for h in range(H)]   # dc^NCH = b^1024
    dci = [[float(_b64[h] ** (C * i)) for i in range(NCH)] for h in range(H)]

    with tile.TileContext(nc) as tc:
        with tc.tile_pool(name="persist", bufs=1) as P, \
             tc.tile_pool(name="tabs", bufs=1) as TB, \
             tc.tile_pool(name="stats", bufs=4) as SP, \
             tc.tile_pool(name="htp", bufs=2) as HTP, \
             tc.tile_pool(name="ps", bufs=6, space="PSUM") as PS, \
             tc.tile_pool(name="ps2", bufs=2, space="PSUM") as PS2, \
             tc.tile_pool(name="dram", bufs=1, space="DRAM") as DR:

            def ps(n=512):
                return PS.tile([128, 512], F32, tag="ps", name="ps")[:, :n]

            def psb(n=512):
                return PS.tile([128, 512], BF, tag="ps", name="psb")[:, :n]

            def ps2(n=512):
                return PS2.tile([128, 512], F32, tag="po", name="po")[:, :n]

            # ---- persistent SBUF ----
            x_sb = P.tile([128, PT, D], BF, name="x_sb")        # x then x2
            hT = P.tile([128, KO, SEG], BF, name="hT")          # h^T (phase 1)

            # ---- DRAM scratch for AllGather (2 groups: h4-7 first, h0-3) --
            _aghs = [(4, 4), (0, 4)]   # (first head, count) per AG group
            cin_h = [DR.tile([128, 4, DV], BF, name=f"cih{i}")
                     for i in range(2)]
            cout_h = [DR.tile([512, 4, DV], BF, name=f"coh{i}")
                      for i in range(2)]

            # ---- tables ----
            dqb = TB.tile([128, H, C], BF, name="dqb")
            nc.sync.dma_start(dqb[:], dqb_d.ap())
            dkcol = TB.tile([128, H, NCH], F32, name="dkcol")
            nc.sync.dma_start(dkcol[:], dkcol_d.ap())
            coefT = TB.tile([128, H, 4], F32, name="coefT")
            nc.sync.dma_start(coefT[:], coef_d.ap())
            b1T = TB.tile([128, FT], F32, name="b1T")
            nc.sync.dma_start(b1T[:], b1T_d.ap())
            eps_t = TB.tile([128, 1], F32, name="eps")
            nc.vector.memset(eps_t[:], EPS)
            eps2_t = TB.tile([128, 1], F32, name="eps2")
            nc.vector.memset(eps2_t[:], EPS / 256.0)
            identb = TB.tile([128, 128], BF, name="identb")
            make_identity(nc, identb[:])

            ns = nc.named_scope

            # ============ LayerNorm + transpose into hT / h8 ============
            # Phase 2 emits 16*h2 in fp8e4 (scale folded into rstd via the
            # sqrt((var+eps)/256) trick); fc1 descales via gelu scale=2^-12.
            def ln_phase(tag, dst_t):
                ph2 = tag == "2"
                for tt in range(PT):
                  with ns(f"ln{tag}_{tt}"):
                    if tag == "1" and tt > 0:   # tt0 DMA hoisted (startup)
                        nc.sync.dma_start(x_sb[:, tt, :], xr[tt])
                    st = SP.tile([128, 2, 6], F32, tag="bst")
                    xg = x_sb[:, tt, :].rearrange("p (s f) -> p s f", f=512)
                    for s in range(2):
                        nc.vector.bn_stats(out=st[:, s, :], in_=xg[:, s, :])
                    mv = SP.tile([128, 2], F32, tag="mv")
                    nc.vector.bn_aggr(out=mv[:], in_=st[:])
                    std = SP.tile([128, 1], F32, tag="std")
                    nc.scalar.activation(std[:], mv[:, 1:2], AF.Sqrt,
                                         bias=eps2_t[:] if ph2 else eps_t[:],
                                         scale=1.0 / 256.0 if ph2 else 1.0)
                    rstd = SP.tile([128, 1], F32, tag="rstd")
                    nc.vector.reciprocal(rstd[:], std[:])
                    nb = SP.tile([128, 1], F32, tag="nb")
                    nc.vector.tensor_scalar(
                        out=nb[:], in0=rstd[:],
                        scalar1=mv[:, 0:1], scalar2=-1.0,
                        op0=ALU.mult, op1=ALU.mult)
                    htmp = HTP.tile([128, D], BF, tag="htmp")
                    nc.scalar.activation(htmp[:], x_sb[:, tt, :], AF.Identity,
                                         bias=nb[:], scale=rstd[:])
                    for g in range(2):
                        pt_ = psb()
                        for k4 in range(4):
                            ko = g * 4 + k4
                            nc.tensor.transpose(
                                pt_[:, k4 * 128:(k4 + 1) * 128],
                                htmp[:, ko * 128:(ko + 1) * 128], identb[:])
                        dst = dst_t[:, g * 4:(g + 1) * 4,
                                    tt * 128:(tt + 1) * 128]
                        src = pt_[:].rearrange("p (k f) -> p k f", f=128)
                        if g == 0 and not ph2:
                            # phase 1 is DVE-bound: split copies across
                            # engines (phase 2's Scalar is the busier one)
                            nc.scalar.copy(dst, src)
                        else:
                            nc.vector.tensor_copy(out=dst, in_=src)

            # ============ per-head retention ============
            with tc.tile_pool(name="retp", bufs=1) as RP, \
                 tc.tile_pool(name="sgp", bufs=1) as SGP, \
                 tc.tile_pool(name="ret", bufs=2) as RET:

              qdqT = RP.tile([128, H, SEG], BF, name="qdqT")    # q*b^(t+1)/sq
              obuf = RP.tile([128, 2, 8, SEG], BF, name="obuf")  # o^T halves
              S = RP.tile([128, H, DV], BF, name="S")           # seg state
              Sb = RP.tile([128, H, DV], BF, name="Sb")         # S_init bf16
              maskT = RP.tile([128, H, 512], BF, name="maskT")

              # startup order: x tile 0 (longest LN chain) first on Sync;
              # the first weight pair in parallel on the GpSimd DMA queue.
              nc.sync.dma_start(x_sb[:, 0, :], xr[0])
              wp2 = RET.tile([128, KO, 1024], BF, tag="wqkv", name="wqkv")
              nc.gpsimd.dma_start(wp2[:], wqkv_d.ap()[2])
              nc.sync.dma_start(maskT[:], maskT_d.ap())

              ln_phase("1", hT)

              _anchor = [None, None]   # last retention [PE matmul, DVE copy]

              def qk_head(h, wp, qT, kT, paired=True):
                hh2 = h % 2
                if paired:
                    # stationary weight reused for both token halves
                    for w0, dstT in ((hh2 * 256, qT), (hh2 * 256 + 128, kT)):
                        pp = [ps(), ps()]
                        for ko in range(KO):
                            for n in range(2):
                                nc.tensor.matmul(
                                    pp[n][:], wp[:, ko, w0:w0 + 128],
                                    hT[:, ko, n * 512:(n + 1) * 512],
                                    start=(ko == 0), stop=(ko == KO - 1))
                        for n in range(2):
                            nc.scalar.copy(dstT[:, n * 512:(n + 1) * 512],
                                           pp[n][:])
                else:
                    # n-outer: the first matmuls need only half of hT, so
                    # the kernel-start pipeline fills the PE sooner
                    for n in range(2):
                        nsl = slice(n * 512, (n + 1) * 512)
                        for w0, dstT in ((hh2 * 256, qT),
                                         (hh2 * 256 + 128, kT)):
                            pq = ps()
                            for ko in range(KO):
                                nc.tensor.matmul(
                                    pq[:], wp[:, ko, w0:w0 + 128],
                                    hT[:, ko, nsl],
                                    start=(ko == 0), stop=(ko == KO - 1))
                            nc.scalar.copy(dstT[:, nsl], pq[:])
                # qdq'_i = q_i * b^(tau+1) * dc^i  (Wq carries 1/sqrt(dk))
                for i in range(NCH):
                    isl = slice(i * 128, (i + 1) * 128)
                    nc.vector.tensor_tensor(out=qdqT[:, h, isl],
                                            in0=qT[:, isl],
                                            in1=dqb[:, h, :], op=ALU.mult)
                    if i > 0:
                        nc.vector.tensor_scalar_mul(
                            out=qdqT[:, h, isl], in0=qdqT[:, h, isl],
                            scalar1=dci[h][i])

              def ret_head(h, qT, kT, vn, vof):
                half, hh = h // 4, h % 4
                # kdk' = k^T-chunk * b^(127-s) * dc^-(i+1)
                kdk = RET.tile([128, NCH, 128], BF, tag="kdk", name="kdk")
                for ig in range(2):
                    pt_ = psb()
                    for k4 in range(4):
                        i = ig * 4 + k4
                        isl = slice(i * 128, (i + 1) * 128)
                        nc.tensor.transpose(pt_[:, k4 * 128:(k4 + 1) * 128],
                                            kT[:, isl], identb[:])
                    for k4 in range(4):
                        i = ig * 4 + k4
                        nc.scalar.activation(
                            kdk[:, i, :], pt_[:, k4 * 128:(k4 + 1) * 128],
                            AF.Copy, scale=dkcol[:, h, i:i + 1])
                # state snapshots T_i = sum_{j<=i} kdk'_j^T v_j -- FIRST, so
                # the segment-final state (feeding the AllGather) is ready
                # before the o-computation fills the PE.
                Ts = RET.tile([128, NCH, DV], BF, tag="Ts", name="Ts")
                for i in range(NCH):
                    px = ps(256)
                    nc.tensor.matmul(px[:], kdk[:, i, :],
                                     vn[:, i, vof:vof + 256],
                                     start=True, stop=True)
                    if i == 0:
                        nc.vector.tensor_copy(out=Ts[:, 0, :], in_=px[:])
                    else:
                        nc.vector.tensor_tensor(out=Ts[:, i, :], in0=px[:],
                                                in1=Ts[:, i - 1, :],
                                                op=ALU.add)
                # segment-final state for AllGather: S = dc^NCH * T_7
                nc.scalar.activation(S[:, h, :], Ts[:, NCH - 1, :],
                                     AF.Copy, scale=dc8[h])
                # A^T quads + mask
                At = RET.tile([128, NCH, 128], BF, tag="At", name="At")
                for ip in range(2):
                    pa = ps()
                    for i4 in range(4):
                        i = ip * 4 + i4
                        isl = slice(i * 128, (i + 1) * 128)
                        nc.tensor.matmul(pa[:, i4 * 128:(i4 + 1) * 128],
                                         kT[:, isl], qT[:, isl],
                                         start=True, stop=True)
                    nc.vector.tensor_tensor(
                        out=At[:, ip * 4:(ip + 1) * 4, :].rearrange(
                            "p k f -> p (k f)"),
                        in0=pa[:], in1=maskT[:, h, :], op=ALU.mult)
                # o^T = (A*mask)^T v + qdq' @ T_(i-1), chunk pairs
                for ip in range(4):
                    po = ps2()
                    for i2 in range(2):
                        i = ip * 2 + i2
                        isl = slice(i * 128, (i + 1) * 128)
                        for j in range(2):
                            col = j * 256 + i2 * 128
                            jsl = slice(vof + j * 128, vof + (j + 1) * 128)
                            mm = nc.tensor.matmul(po[:, col:col + 128],
                                                  vn[:, i, jsl], At[:, i, :],
                                                  start=True, stop=(i == 0))
                            if i > 0:
                                mm = nc.tensor.matmul(
                                    po[:, col:col + 128],
                                    Ts[:, i - 1, j * 128:(j + 1) * 128],
                                    qdqT[:, h, isl],
                                    start=False, stop=True)
                            _anchor[0] = mm
                    dst = obuf[:, half, hh * 2:hh * 2 + 2,
                               ip * 256:(ip + 1) * 256]
                    _anchor[1] = nc.vector.tensor_copy(
                        out=dst,
                        in_=po[:].rearrange("p (j f) -> p j f", f=256))

              def calc_vn(a, wp):
                # v for both heads: [s, v_a(256)|v_b(256)]
                with ns(f"head{a}"):
                    vn = RET.tile([128, NCH, 512], BF, tag="vn", name="vn")
                    for i in range(NCH):
                        isl = slice(i * 128, (i + 1) * 128)
                        pv = ps()
                        for ko in range(KO):
                            nc.tensor.matmul(pv[:], hT[:, ko, isl],
                                             wp[:, ko, 512:1024],
                                             start=(ko == 0),
                                             stop=(ko == KO - 1))
                        nc.scalar.copy(vn[:, i, :], pv[:])
                return vn

              def pair(p, wp=None, v_first=False):
                a, b = 2 * p, 2 * p + 1
                # v_first: v chunk i only needs hT tile i, so at kernel start
                # the PE can begin before the whole LN phase has finished
                # (the paired q/k matmuls touch all of hT at once).
                if v_first:
                    vn = calc_vn(a, wp)
                with ns(f"head{a}"):
                    if wp is None:
                        wp = RET.tile([128, KO, 1024], BF, tag="wqkv",
                                      name="wqkv")
                        nc.sync.dma_start(wp[:], wqkv_d.ap()[p])
                    qTa = RET.tile([128, SEG], BF, tag="qT", name="qT")
                    kTa = RET.tile([128, SEG], BF, tag="kT", name="kT")
                    qk_head(a, wp, qTa, kTa, paired=not v_first)
                with ns(f"head{b}"):
                    qTb = RET.tile([128, SEG], BF, tag="qT", name="qT")
                    kTb = RET.tile([128, SEG], BF, tag="kT", name="kT")
                    qk_head(b, wp, qTb, kTb)
                if not v_first:
                    vn = calc_vn(a, wp)
                with ns(f"head{a}"):
                    ret_head(a, qTa, kTa, vn, 0)
                with ns(f"head{b}"):
                    ret_head(b, qTb, kTb, vn, 256)

              _sgg = {}

              def ag(gi):
                h0, cnt = _aghs[gi]
                with ns(f"ag{gi}"):
                    # staging DMA on the Sync queue: the GpSimd queue's
                    # instruction guards dispatch ~14us late around the
                    # collective triggers
                    nc.sync.dma_start(cin_h[gi][:], S[:, h0:h0 + cnt, :])
                    nc.gpsimd.collective_compute(
                        "AllGather", ALU.bypass,
                        replica_groups=[[0, 1, 2, 3], [4, 5, 6, 7]],
                        ins=[cin_h[gi].opt()], outs=[cout_h[gi].opt()])
                    # gather the whole group's states in ONE DMA (a tile per
                    # group: no buffer aliasing, no WAR chain on the queue)
                    coutv = cout_h[gi][:].rearrange(
                        "(j p) h v -> p j h v", p=128)
                    Sgg = SGP.tile([128, 4, 4, DV], BF, tag="Sgg", name="Sgg")
                    nc.gpsimd.dma_start(Sgg[:], coutv)
                    _sgg[gi] = Sgg

              # halves 4-7 first: their AG overlaps heads 0-3; the final AG
              # (heads 0-3) hides under corr4-7 + wo1.
              pair(2, wp2, v_first=True)
              pair(3)
              ag(0)
              pair(0)
              pair(1)
              ag(1)

              # ---- S_init per head (post-AG) on GpSimd DMA + DVE math,
              # issued after all retention DVE work and pinned behind it
              # (the in-order DVE queue would otherwise stall retention
              # behind the collective-gated ops) ----
              def sinit(h):
                gi = 0 if h >= 4 else 1
                h0, cnt = _aghs[gi]
                with ns(f"sinit{h}"):
                    Sg = _sgg[gi]
                    Si = Sb[:, h, :]
                    op0 = nc.vector.tensor_scalar_mul(
                        out=Si, in0=Sg[:, 0, h - h0, :],
                        scalar1=coefT[:, h, 0:1])
                    if _anchor[1] is not None:
                        add_dep_helper(op0.ins, _anchor[1].ins, sync=False,
                                       reason="sinit after retention DVE")
                    for j in range(1, 4):
                        nc.vector.scalar_tensor_tensor(
                            out=Si, in0=Sg[:, j, h - h0, :],
                            scalar=coefT[:, h, j:j + 1], in1=Si,
                            op0=ALU.mult, op1=ALU.add)

              # ---- correction: obuf += (qdq' @ S_init)^T ----
              def corr(h):
                half, hh = h // 4, h % 4
                with ns(f"corr{h}"):
                    for n in range(2):
                        nsl = slice(n * 512, (n + 1) * 512)
                        for j in range(2):
                            pc = ps()
                            mm = nc.tensor.matmul(
                                pc[:], Sb[:, h, j * 128:(j + 1) * 128],
                                qdqT[:, h, nsl], start=True, stop=True)
                            if _anchor[0] is not None:
                                add_dep_helper(mm.ins, _anchor[0].ins,
                                               sync=False,
                                               reason="corr after retention")
                            dst = obuf[:, half, hh * 2 + j, nsl]
                            nc.vector.tensor_tensor(out=dst, in0=pc[:],
                                                    in1=dst, op=ALU.add)

              # ---- Wo ----
              def wo_round(half):
                wons = []
                for n in range(2):
                    won = RET.tile([128, 8, 512], BF, tag="won", name="won")
                    nc.sync.dma_start(won[:], wo_d.ap()[half, :, n])
                    wons.append(won)
                for tt in range(PT):
                    tsl = slice(tt * 128, (tt + 1) * 128)
                    with ns(f"wo{half}_{tt}"):
                        pw = [ps(), ps()]
                        for r in range(8):     # stationary reused for both n
                            for n in range(2):
                                nc.tensor.matmul(
                                    pw[n][:], obuf[:, half, r, tsl],
                                    wons[n][:, r, :],
                                    start=(r == 0), stop=(r == 7))
                        for n in range(2):
                            nsl = slice(n * 512, (n + 1) * 512)
                            nc.vector.tensor_tensor(out=x_sb[:, tt, nsl],
                                                    in0=pw[n][:],
                                                    in1=x_sb[:, tt, nsl],
                                                    op=ALU.add)

              for h in range(4, 8):
                  sinit(h)
                  corr(h)
              wo_round(1)
              for h in range(4):
                  sinit(h)
                  corr(h)
              wo_round(0)

            # ============ LN2 + FFN (x_sb now holds x2) ============
            with tc.tile_pool(name="ffn", bufs=2) as FP, \
                 tc.tile_pool(name="yap", bufs=1) as YA:
                h8 = YA.tile([128, KO, SEG], F8, name="h8")     # 16*h2^T fp8
                ln_phase("2", h8)
                y_acc = YA.tile([128, PT, D], F32, name="yacc")
                for g in range(4):
                    w1g = FP.tile([128, 8, KO, 128], F8, tag="w1g",
                                  name="w1g")
                    nc.sync.dma_start(w1g[:], w1_d.ap()[g])
                    w2g = FP.tile([128, 8, 1024], F8, tag="w2g", name="w2g")
                    nc.sync.dma_start(w2g[:], w2_d.ap()[g])
                    fT = FP.tile([128, 8, 2, 512], F8, tag="fT", name="fT")
                    for kt in range(8):
                      with ns(f"ffn{g}_f{kt}"):
                        ktr = g * 8 + kt
                        pf = [ps(), ps()]
                        for k2 in range(KO // 2):
                            k2s = slice(2 * k2, 2 * k2 + 2)
                            for th in range(2):  # stationary reused
                                nc.tensor.matmul(
                                    pf[th][:], w1g[:, kt, k2s, :],
                                    h8[:, k2s, th * 512:(th + 1) * 512],
                                    start=(k2 == 0),
                                    stop=(k2 == KO // 2 - 1),
                                    perf_mode=PM.DoubleRow)
                        for th in range(2):
                            nc.scalar.activation(fT[:, kt, th, :], pf[th][:],
                                                 AF.Gelu,
                                                 bias=b1T[:, ktr:ktr + 1],
                                                 scale=1.0 / 4096.0)
                    for th in range(2):
                        for tt4 in range(4):
                            tt = th * 4 + tt4
                            t4 = slice(tt4 * 128, (tt4 + 1) * 128)
                            for n in range(2):
                              with ns(f"ffn{g}_{th}_y{tt4}_{n}"):
                                nsl = slice(n * 512, (n + 1) * 512)
                                pg = ps2()
                                for k2 in range(4):
                                    k2s = slice(2 * k2, 2 * k2 + 2)
                                    nc.tensor.matmul(
                                        pg[:], fT[:, k2s, th, t4],
                                        w2g[:, k2s, nsl],
                                        start=(k2 == 0), stop=(k2 == 3),
                                        perf_mode=PM.DoubleRow)
                                if g == 0:
                                    nc.vector.tensor_tensor(
                                        out=y_acc[:, tt, nsl], in0=pg[:],
                                        in1=x_sb[:, tt, nsl], op=ALU.add)
                                elif g < 3:
                                    nc.vector.tensor_tensor(
                                        out=y_acc[:, tt, nsl], in0=pg[:],
                                        in1=y_acc[:, tt, nsl], op=ALU.add)
                                else:
                                    yt = FP.tile([128, 512], BF, tag="yt",
                                                 name="yt")
                                    nc.vector.tensor_tensor(
                                        out=yt[:], in0=pg[:],
                                        in1=y_acc[:, tt, nsl], op=ALU.add)
                                    nc.sync.dma_start(yr[tt][:, nsl], yt[:])

    nc.compile()
    return nc


def _host_prep(inputs):
    x = np.asarray(inputs["x"], np.float32)
    ln1_w = np.asarray(inputs["ln1_w"], np.float32)
    ln1_b = np.asarray(inputs["ln1_b"], np.float32)
    Wq = np.asarray(inputs["Wq"], np.float32)
    Wk = np.asarray(inputs["Wk"], np.float32)
    Wv = np.asarray(inputs["Wv"], np.float32)
    Wo = np.asarray(inputs["Wo"], np.float32)
    ln2_w = np.asarray(inputs["ln2_w"], np.float32)
    ln2_b = np.asarray(inputs["ln2_b"], np.float32)
    W1 = np.asarray(inputs["W1"], np.float32)
    b1 = np.asarray(inputs["b1"], np.float32)
    W2 = np.asarray(inputs["W2"], np.float32)
    b2 = np.asarray(inputs["b2"], np.float32)

    assert np.all(ln1_b == 0) and np.all(ln2_b == 0) and np.all(b2 == 0), \
        "kernel build assumes zero ln1_b/ln2_b/b2 (gated paths not emitted)"

    sc = 1.0 / np.sqrt(np.float64(DK))
    wq_e = ln1_w[:, None] * Wq * sc     # fold 1/sqrt(dk) into Wq
    wk_e = ln1_w[:, None] * Wk
    wv_e = ln1_w[:, None] * Wv
    w1_e = ln2_w[:, None] * W1

    def bf(a):
        return np.ascontiguousarray(a).astype(BF_NP)

    def f8(a):
        return np.ascontiguousarray(np.clip(a, -240.0, 240.0)).astype(F8_NP)

    wqh = wq_e.reshape(KO, 128, H, 128).transpose(2, 1, 0, 3)
    wkh = wk_e.reshape(KO, 128, H, 128).transpose(2, 1, 0, 3)
    wvh = wv_e.reshape(KO, 128, H, 256).transpose(2, 1, 0, 3)
    # per head-pair: [qa | ka | qb | kb | va | vb] -> [4, 128, KO, 1024]
    wqkv = bf(np.concatenate(
        [np.stack([wqh[0::2], wkh[0::2], wqh[1::2], wkh[1::2]], axis=0)
         .transpose(1, 2, 3, 0, 4).reshape(4, 128, KO, 512),
         np.concatenate([wvh[0::2], wvh[1::2]], axis=-1)], axis=-1))

    wo2 = bf(Wo.reshape(2, 8, 128, 2, 512).transpose(0, 2, 3, 1, 4))
    w1t = f8((w1_e * 256.0).reshape(KO, 128, FT, 128).transpose(2, 1, 0, 3)
             .reshape(4, 8, 128, KO, 128).transpose(0, 2, 1, 3, 4))
    w2t = f8(W2.reshape(4, 8, 128, 1024).transpose(0, 2, 1, 3))

    t_ = np.arange(C, dtype=np.float64)
    maskT2 = np.zeros((128, H, 512), np.float64)
    dqbf = np.zeros((128, H, C), np.float64)
    dkcol2 = np.zeros((128, H, NCH), np.float32)
    for h in range(H):
        diff = t_[None, :] - t_[:, None]      # [s, t] -> t - s
        m = np.where(diff >= 0, np.exp(_logb[h] * diff), 0.0)
        for r4 in range(4):
            maskT2[:, h, r4 * 128:(r4 + 1) * 128] = m
        dqbf[:, h, :] = np.exp(_logb[h] * (t_ + 1.0))[None, :]
        dc = np.exp(_logb[h] * C)
        for i in range(NCH):
            dkcol2[:, h, i] = (np.exp(_logb[h] * (C - 1.0 - t_))
                               * dc ** (-(i + 1.0)))
    maskT2 = bf(maskT2)
    dqbf = bf(dqbf)

    b1_e = b1 + ln2_b @ W1
    b1T = np.ascontiguousarray(b1_e.reshape(FT, 128).T).astype(np.float32)

    shared = dict(wqkv=wqkv, wo2=wo2, w1t=w1t, w2t=w2t,
                  maskT2=maskT2, dqbf=dqbf, dkcol2=dkcol2, b1T=b1T)

    in_maps = []
    for c in range(NCORES):
        b, s = c // 4, c % 4
        coefT = np.zeros((128, H, 4), np.float32)
        for h in range(H):
            for j in range(4):
                if j < s:
                    coefT[:, h, j] = np.exp(_logb[h] * (SEG * (s - 1 - j)))
        m = dict(shared)
        m["x"] = bf(x[b, s * SEG:(s + 1) * SEG, :])
        m["coefT"] = coefT
        in_maps.append(m)
    return in_maps


def kernel(**inputs):
    if "nc" not in _NC_CACHE:
        _NC_CACHE["nc"] = _build()
    nc = _NC_CACHE["nc"]
    in_maps = _host_prep(inputs)
    res = run_bass_kernel_spmd(nc, in_maps, core_ids=list(range(NCORES)))
    _NC_CACHE["last_res"] = res
    out = np.zeros((B, L, D), np.float32)
    for c in range(NCORES):
        b, s = c // 4, c % 4
        out[b, s * SEG:(s + 1) * SEG, :] = res.results[c]["y"]
    return out



# revision 43
# speedup vs baseline: 1.0346x; 1.0151x over previous
"""Trainium2 Bass kernel v2 for nn_Block_85126251807269 (RetNet-style block).

Sharding: token-parallel over 8 NeuronCores (core c -> batch c//4, tokens
[1024*(c%4), 1024*(c%4+1))). Weights replicated (bf16). Cross-core comm is
one AllGather of per-head retention segment states (512KB/core) per half.

Key differences vs v1:
  - bf16 matmul operands everywhere (fp32 PSUM accumulation): 1 cyc/row
    even for the small N=128 retention matmuls (fp32 was 4 cyc/row), and
    half the weight DMA traffic.
  - Retention state chain reformulated: with kdk'_i = k_i*b^(C-1-s)*dc^-(i+1)
    and qdq'_i = q_i*b^(t+1)/sqrt(dk) (t global in segment), the state is a
    pure sum T_i = sum_j K_j (K_j = kdk'_j^T v_j), so there is no per-chunk
    scale+add serial chain; per-chunk state snapshots are single DVE adds.
  - x2 and qdq kept in SBUF (no DRAM roundtrips); ~60 large DMAs instead of
    ~780 small ones.
  - FFN: y accumulated in SBUF fp32 across 4 kt-groups; PSUM banks released
    per (tt,n) so fc1/fc2 pipeline without bank deadlocks.
  - Cross-core correction o += qdq' @ S_init applied directly to obuf
    (qdq' already carries the dc^i chunk scaling, so no rescale needed).
"""
import sys

sys.path.insert(0, "/opt/trn_rl_repo")
import antenv  # noqa: E402

if not hasattr(antenv, "axon_hooks"):
    import importlib.util

    _spec = importlib.util.spec_from_file_location(
        "antenv.axon_hooks", "/opt/trn_rl_repo/antenv/axon_hooks.py")
    if _spec is not None and _spec.loader is not None:
        try:
            _m = importlib.util.module_from_spec(_spec)
            _spec.loader.exec_module(_m)
            sys.modules["antenv.axon_hooks"] = _m
            antenv.axon_hooks = _m
        except Exception:
            pass

import numpy as np  # noqa: E402
import ml_dtypes  # noqa: E402
import concourse.bacc as bacc  # noqa: E402
import concourse.mybir as mybir  # noqa: E402
import concourse.tile as tile  # noqa: E402
from concourse.bass_utils import run_bass_kernel_spmd  # noqa: E402
from concourse.masks import make_identity  # noqa: E402
from concourse.tile_rust import add_dep_helper  # noqa: E402

dt = mybir.dt
AF = mybir.ActivationFunctionType
ALU = mybir.AluOpType
PM = mybir.MatmulPerfMode
BF = dt.bfloat16
F32 = dt.float32
F8 = dt.float8e4
BF_NP = ml_dtypes.bfloat16
F8_NP = ml_dtypes.float8_e4m3

B, L, D, H = 2, 4096, 1024, 8
DK, DV, FFN = 128, 256, 4096
NCORES = 8
SEG = 1024          # tokens per core
PT = SEG // 128     # token tiles per core
C = 128             # retention chunk
NCH = SEG // C      # chunks per core
KO = D // 128       # contraction tiles over D
FT = FFN // 128     # ffn col tiles
EPS = 1e-6

_b64 = (1.0 - np.exp2(-5.0 - np.arange(H))).astype(np.float64)
_logb = np.log(_b64)

_NC_CACHE = {}


def _build():
    nc = bacc.Bacc("TRN2", target_bir_lowering=False, debug=False,
                   num_devices=NCORES)

    def din(name, shape, d=F32):
        return nc.dram_tensor(name, list(shape), d, kind="ExternalInput")

    x_d = din("x", [SEG, D], BF)
    wqkv_d = din("wqkv", [4, 128, KO, 1024], BF)  # per head-pair:
    # cols = [q_a(128) | k_a(128) | q_b(128) | k_b(128) | v_a(256) | v_b(256)]
    wo_d = din("wo2", [2, 128, 2, 8, 512], BF)    # [half,p,n,r,c]; r=hh*2+j
    w1_d = din("w1t", [4, 128, 8, KO, 128], F8)   # [g,p,kt,ko,m] (x256)
    w2_d = din("w2t", [4, 128, 8, 1024], F8)      # [g,p,kt,c]
    maskT_d = din("maskT2", [128, H, 512], BF)    # [s,h,(4,t)] decayed mask^T
    dqb_d = din("dqbf", [128, H, C], BF)          # [p,h,tau] = b^(tau+1)
    dkcol_d = din("dkcol2", [128, H, NCH])        # [s,h,i]=b^(127-s)*dc^-(i+1)
    coef_d = din("coefT", [128, H, 4])            # per-core prefix coefs
    b1T_d = din("b1T", [128, FT])

    y_d = nc.dram_tensor("y", [SEG, D], BF, kind="ExternalOutput")
    yr = y_d.ap().rearrange("(tt p) d -> tt p d", p=128)
    xr = x_d.ap().rearrange("(tt p) d -> tt p d", p=128)

    dc8 = [float(_b64[h] ** SEG) for h in range(H)]   # dc^NCH = b^1024
    dci = [[float(_b64[h] ** (C * i)) for i in range(NCH)] for h in range(H)]

    with tile.TileContext(nc) as tc:
        with tc.tile_pool(name="persist", bufs=1) as P, \
             tc.tile_pool(name="tabs", bufs=1) as TB, \
             tc.tile_pool(name="stats", bufs=4) as SP, \
             tc.tile_pool(name="htp", bufs=2) as HTP, \
             tc.tile_pool(name="ps", bufs=4, space="PSUM") as PS, \
             tc.tile_pool(name="psq", bufs=2, space="PSUM") as PSQ, \
             tc.tile_pool(name="ps2", bufs=2, space="PSUM") as PS2, \
             tc.tile_pool(name="dram", bufs=1, space="DRAM") as DR:

            def ps(n=512):
                return PS.tile([128, 512], F32, tag="ps", name="ps")[:, :n]

            def psb(n=512):
                return PS.tile([128, 512], BF, tag="ps", name="psb")[:, :n]

            def psq(n=512):
                # long-held paired accumulators (qk/wo/fc1): own pool so
                # they don't starve the round-robin ps() ring
                return PSQ.tile([128, 512], F32, tag="pq", name="pq")[:, :n]

            def ps2(n=512):
                return PS2.tile([128, 512], F32, tag="po", name="po")[:, :n]

            # ---- persistent SBUF ----
            x_sb = P.tile([128, PT, D], BF, name="x_sb")        # x then x2
            hT = P.tile([128, KO, SEG], BF, name="hT")          # h^T (phase 1)

            # ---- DRAM scratch for AllGather (2 groups: h4-7 first, h0-3) --
            _aghs = [(4, 4), (0, 4)]   # (first head, count) per AG group
            cin_h = [DR.tile([128, 4, DV], BF, name=f"cih{i}")
                     for i in range(2)]
            cout_h = [DR.tile([512, 4, DV], BF, name=f"coh{i}")
                      for i in range(2)]

            # ---- tables ----
            dqb = TB.tile([128, H, C], BF, name="dqb")
            nc.sync.dma_start(dqb[:], dqb_d.ap())
            dkcol = TB.tile([128, H, NCH], F32, name="dkcol")
            nc.sync.dma_start(dkcol[:], dkcol_d.ap())
            coefT = TB.tile([128, H, 4], F32, name="coefT")
            nc.sync.dma_start(coefT[:], coef_d.ap())
            b1T = TB.tile([128, FT], F32, name="b1T")
            nc.sync.dma_start(b1T[:], b1T_d.ap())
            eps_t = TB.tile([128, 1], F32, name="eps")
            nc.vector.memset(eps_t[:], EPS)
            eps2_t = TB.tile([128, 1], F32, name="eps2")
            nc.vector.memset(eps2_t[:], EPS / 256.0)
            identb = TB.tile([128, 128], BF, name="identb")
            make_identity(nc, identb[:])

            ns = nc.named_scope

            # ============ LayerNorm + transpose into hT / h8 ============
            # Phase 2 emits 16*h2 in fp8e4 (scale folded into rstd via the
            # sqrt((var+eps)/256) trick); fc1 descales via gelu scale=2^-12.
            def ln_phase(tag, dst_t):
                ph2 = tag == "2"
                for tt in range(PT):
                  with ns(f"ln{tag}_{tt}"):
                    if tag == "1" and tt > 0:   # tt0 DMA hoisted (startup)
                        nc.sync.dma_start(x_sb[:, tt, :], xr[tt])
                    st = SP.tile([128, 2, 6], F32, tag="bst")
                    xg = x_sb[:, tt, :].rearrange("p (s f) -> p s f", f=512)
                    for s in range(2):
                        nc.vector.bn_stats(out=st[:, s, :], in_=xg[:, s, :])
                    mv = SP.tile([128, 2], F32, tag="mv")
                    nc.vector.bn_aggr(out=mv[:], in_=st[:])
                    std = SP.tile([128, 1], F32, tag="std")
                    nc.scalar.activation(std[:], mv[:, 1:2], AF.Sqrt,
                                         bias=eps2_t[:] if ph2 else eps_t[:],
                                         scale=1.0 / 256.0 if ph2 else 1.0)
                    rstd = SP.tile([128, 1], F32, tag="rstd")
                    nc.vector.reciprocal(rstd[:], std[:])
                    nb = SP.tile([128, 1], F32, tag="nb")
                    nc.vector.tensor_scalar(
                        out=nb[:], in0=rstd[:],
                        scalar1=mv[:, 0:1], scalar2=-1.0,
                        op0=ALU.mult, op1=ALU.mult)
                    htmp = HTP.tile([128, D], BF, tag="htmp")
                    nc.scalar.activation(htmp[:], x_sb[:, tt, :], AF.Identity,
                                         bias=nb[:], scale=rstd[:])
                    for g in range(2):
                        pt_ = psb()
                        for k4 in range(4):
                            ko = g * 4 + k4
                            nc.tensor.transpose(
                                pt_[:, k4 * 128:(k4 + 1) * 128],
                                htmp[:, ko * 128:(ko + 1) * 128], identb[:])
                        dst = dst_t[:, g * 4:(g + 1) * 4,
                                    tt * 128:(tt + 1) * 128]
                        src = pt_[:].rearrange("p (k f) -> p k f", f=128)
                        if g == 0 and not ph2:
                            # phase 1 is DVE-bound: split copies across
                            # engines (phase 2's Scalar is the busier one)
                            nc.scalar.copy(dst, src)
                        else:
                            nc.vector.tensor_copy(out=dst, in_=src)

            # ============ per-head retention ============
            with tc.tile_pool(name="retp", bufs=1) as RP, \
                 tc.tile_pool(name="sgp", bufs=1) as SGP, \
                 tc.tile_pool(name="ret", bufs=2) as RET:

              qdqT = RP.tile([128, H, SEG], BF, name="qdqT")    # q*b^(t+1)/sq
              obuf = RP.tile([128, 2, 8, SEG], BF, name="obuf")  # o^T halves
              S = RP.tile([128, H, DV], BF, name="S")           # seg state
              Sb = RP.tile([128, H, DV], BF, name="Sb")         # S_init bf16
              maskT = RP.tile([128, H, 512], BF, name="maskT")

              # startup order: x tile 0 (longest LN chain) first on Sync;
              # the first weight pair in parallel on the GpSimd DMA queue.
              nc.sync.dma_start(x_sb[:, 0, :], xr[0])
              wp2 = RET.tile([128, KO, 1024], BF, tag="wqkv", name="wqkv")
              nc.gpsimd.dma_start(wp2[:], wqkv_d.ap()[2])
              nc.sync.dma_start(maskT[:], maskT_d.ap())

              ln_phase("1", hT)

              _anchor = [None, None]   # last retention [PE matmul, DVE copy]

              def qk_head(h, wp, qT, kT, paired=True):
                hh2 = h % 2
                if paired:
                    # stationary weight reused for both token halves
                    for w0, dstT in ((hh2 * 256, qT), (hh2 * 256 + 128, kT)):
                        pp = [psq(), psq()]
                        for ko in range(KO):
                            for n in range(2):
                                nc.tensor.matmul(
                                    pp[n][:], wp[:, ko, w0:w0 + 128],
                                    hT[:, ko, n * 512:(n + 1) * 512],
                                    start=(ko == 0), stop=(ko == KO - 1))
                        for n in range(2):
                            nc.scalar.copy(dstT[:, n * 512:(n + 1) * 512],
                                           pp[n][:])
                else:
                    # n-outer: the first matmuls need only half of hT, so
                    # the kernel-start pipeline fills the PE sooner
                    for n in range(2):
                        nsl = slice(n * 512, (n + 1) * 512)
                        for w0, dstT in ((hh2 * 256, qT),
                                         (hh2 * 256 + 128, kT)):
                            pq = ps()
                            for ko in range(KO):
                                nc.tensor.matmul(
                                    pq[:], wp[:, ko, w0:w0 + 128],
                                    hT[:, ko, nsl],
                                    start=(ko == 0), stop=(ko == KO - 1))
                            nc.scalar.copy(dstT[:, nsl], pq[:])
                # qdq'_i = q_i * b^(tau+1) * dc^i  (Wq carries 1/sqrt(dk))
                for i in range(NCH):
                    isl = slice(i * 128, (i + 1) * 128)
                    nc.vector.tensor_tensor(out=qdqT[:, h, isl],
                                            in0=qT[:, isl],
                                            in1=dqb[:, h, :], op=ALU.mult)
                    if i > 0:
                        nc.vector.tensor_scalar_mul(
                            out=qdqT[:, h, isl], in0=qdqT[:, h, isl],
                            scalar1=dci[h][i])

              def ret_head(h, qT, kT, vn, vof):
                half, hh = h // 4, h % 4
                # kdk' = k^T-chunk * b^(127-s) * dc^-(i+1)
                kdk = RET.tile([128, NCH, 128], BF, tag="kdk", name="kdk")
                for ig in range(2):
                    pt_ = psb()
                    for k4 in range(4):
                        i = ig * 4 + k4
                        isl = slice(i * 128, (i + 1) * 128)
                        nc.tensor.transpose(pt_[:, k4 * 128:(k4 + 1) * 128],
                                            kT[:, isl], identb[:])
                    for k4 in range(4):
                        i = ig * 4 + k4
                        nc.scalar.activation(
                            kdk[:, i, :], pt_[:, k4 * 128:(k4 + 1) * 128],
                            AF.Copy, scale=dkcol[:, h, i:i + 1])
                # A^T quads + mask
                At = RET.tile([128, NCH, 128], BF, tag="At", name="At")
                for ip in range(2):
                    pa = ps()
                    for i4 in range(4):
                        i = ip * 4 + i4
                        isl = slice(i * 128, (i + 1) * 128)
                        nc.tensor.matmul(pa[:, i4 * 128:(i4 + 1) * 128],
                                         kT[:, isl], qT[:, isl],
                                         start=True, stop=True)
                    nc.vector.tensor_tensor(
                        out=At[:, ip * 4:(ip + 1) * 4, :].rearrange(
                            "p k f -> p (k f)"),
                        in0=pa[:], in1=maskT[:, h, :], op=ALU.mult)
                # state snapshots T_i = sum_{j<=i} kdk'_j^T v_j
                Ts = RET.tile([128, NCH, DV], BF, tag="Ts", name="Ts")
                for i in range(NCH):
                    px = ps(256)
                    nc.tensor.matmul(px[:], kdk[:, i, :],
                                     vn[:, i, vof:vof + 256],
                                     start=True, stop=True)
                    if i == 0:
                        nc.vector.tensor_copy(out=Ts[:, 0, :], in_=px[:])
                    else:
                        nc.vector.tensor_tensor(out=Ts[:, i, :], in0=px[:],
                                                in1=Ts[:, i - 1, :],
                                                op=ALU.add)
                # segment-final state (feeds the AllGather) right after the
                # Ts chain, before the o-matmuls
                nc.scalar.activation(S[:, h, :], Ts[:, NCH - 1, :],
                                     AF.Copy, scale=dc8[h])
                # o^T = (A*mask)^T v + qdq' @ T_(i-1), chunk pairs
                for ip in range(4):
                    po = ps2()
                    for i2 in range(2):
                        i = ip * 2 + i2
                        isl = slice(i * 128, (i + 1) * 128)
                        for j in range(2):
                            col = j * 256 + i2 * 128
                            jsl = slice(vof + j * 128, vof + (j + 1) * 128)
                            mm = nc.tensor.matmul(po[:, col:col + 128],
                                                  vn[:, i, jsl], At[:, i, :],
                                                  start=True, stop=(i == 0))
                            if i > 0:
                                mm = nc.tensor.matmul(
                                    po[:, col:col + 128],
                                    Ts[:, i - 1, j * 128:(j + 1) * 128],
                                    qdqT[:, h, isl],
                                    start=False, stop=True)
                            _anchor[0] = mm
                    dst = obuf[:, half, hh * 2:hh * 2 + 2,
                               ip * 256:(ip + 1) * 256]
                    _anchor[1] = nc.vector.tensor_copy(
                        out=dst,
                        in_=po[:].rearrange("p (j f) -> p j f", f=256))

              def calc_vn(a, wp):
                # v for both heads: [s, v_a(256)|v_b(256)]
                with ns(f"head{a}"):
                    vn = RET.tile([128, NCH, 512], BF, tag="vn", name="vn")
                    for i in range(NCH):
                        isl = slice(i * 128, (i + 1) * 128)
                        pv = ps()
                        for ko in range(KO):
                            nc.tensor.matmul(pv[:], hT[:, ko, isl],
                                             wp[:, ko, 512:1024],
                                             start=(ko == 0),
                                             stop=(ko == KO - 1))
                        nc.scalar.copy(vn[:, i, :], pv[:])
                return vn

              def pair(p, wp=None, v_first=False):
                a, b = 2 * p, 2 * p + 1
                # v_first: v chunk i only needs hT tile i, so at kernel start
                # the PE can begin before the whole LN phase has finished
                # (the paired q/k matmuls touch all of hT at once).
                if v_first:
                    vn = calc_vn(a, wp)
                with ns(f"head{a}"):
                    if wp is None:
                        wp = RET.tile([128, KO, 1024], BF, tag="wqkv",
                                      name="wqkv")
                        nc.sync.dma_start(wp[:], wqkv_d.ap()[p])
                    qTa = RET.tile([128, SEG], BF, tag="qT", name="qT")
                    kTa = RET.tile([128, SEG], BF, tag="kT", name="kT")
                    qk_head(a, wp, qTa, kTa, paired=not v_first)
                with ns(f"head{b}"):
                    qTb = RET.tile([128, SEG], BF, tag="qT", name="qT")
                    kTb = RET.tile([128, SEG], BF, tag="kT", name="kT")
                    qk_head(b, wp, qTb, kTb)
                if not v_first:
                    vn = calc_vn(a, wp)
                with ns(f"head{a}"):
                    ret_head(a, qTa, kTa, vn, 0)
                with ns(f"head{b}"):
                    ret_head(b, qTb, kTb, vn, 256)

              _sgg = {}

              def ag(gi):
                h0, cnt = _aghs[gi]
                with ns(f"ag{gi}"):
                    # staging DMA on the Sync queue: the GpSimd queue's
                    # instruction guards dispatch ~14us late around the
                    # collective triggers
                    nc.sync.dma_start(cin_h[gi][:], S[:, h0:h0 + cnt, :])
                    nc.gpsimd.collective_compute(
                        "AllGather", ALU.bypass,
                        replica_groups=[[0, 1, 2, 3], [4, 5, 6, 7]],
                        ins=[cin_h[gi].opt()], outs=[cout_h[gi].opt()])
                    # gather the whole group's states in ONE DMA (a tile per
                    # group: no buffer aliasing, no WAR chain on the queue)
                    coutv = cout_h[gi][:].rearrange(
                        "(j p) h v -> p j h v", p=128)
                    Sgg = SGP.tile([128, 4, 4, DV], BF, tag="Sgg", name="Sgg")
                    nc.gpsimd.dma_start(Sgg[:], coutv)
                    _sgg[gi] = Sgg

              # halves 4-7 first: their AG overlaps heads 0-3; the final AG
              # (heads 0-3) hides under corr4-7 + wo1.
              pair(2, wp2, v_first=True)
              pair(3)
              ag(0)
              pair(0)
              pair(1)
              ag(1)

              # ---- S_init per head (post-AG) on GpSimd DMA + DVE math,
              # issued after all retention DVE work and pinned behind it
              # (the in-order DVE queue would otherwise stall retention
              # behind the collective-gated ops) ----
              def sinit(h):
                gi = 0 if h >= 4 else 1
                h0, cnt = _aghs[gi]
                with ns(f"sinit{h}"):
                    Sg = _sgg[gi]
                    Si = Sb[:, h, :]
                    op0 = nc.vector.tensor_scalar_mul(
                        out=Si, in0=Sg[:, 0, h - h0, :],
                        scalar1=coefT[:, h, 0:1])
                    if _anchor[1] is not None:
                        add_dep_helper(op0.ins, _anchor[1].ins, sync=False,
                                       reason="sinit after retention DVE")
                    for j in range(1, 4):
                        nc.vector.scalar_tensor_tensor(
                            out=Si, in0=Sg[:, j, h - h0, :],
                            scalar=coefT[:, h, j:j + 1], in1=Si,
                            op0=ALU.mult, op1=ALU.add)

              # ---- correction: obuf += (qdq' @ S_init)^T ----
              def corr(h):
                half, hh = h // 4, h % 4
                with ns(f"corr{h}"):
                    for n in range(2):
                        nsl = slice(n * 512, (n + 1) * 512)
                        for j in range(2):
                            pc = ps()
                            mm = nc.tensor.matmul(
                                pc[:], Sb[:, h, j * 128:(j + 1) * 128],
                                qdqT[:, h, nsl], start=True, stop=True)
                            if _anchor[0] is not None:
                                add_dep_helper(mm.ins, _anchor[0].ins,
                                               sync=False,
                                               reason="corr after retention")
                            dst = obuf[:, half, hh * 2 + j, nsl]
                            nc.vector.tensor_tensor(out=dst, in0=pc[:],
                                                    in1=dst, op=ALU.add)

              # ---- Wo ----
              def wo_round(half):
                wons = []
                for n in range(2):
                    won = RET.tile([128, 8, 512], BF, tag="won", name="won")
                    nc.sync.dma_start(won[:], wo_d.ap()[half, :, n])
                    wons.append(won)
                for tt in range(PT):
                    tsl = slice(tt * 128, (tt + 1) * 128)
                    with ns(f"wo{half}_{tt}"):
                        pw = [psq(), psq()]
                        for r in range(8):     # stationary reused for both n
                            for n in range(2):
                                nc.tensor.matmul(
                                    pw[n][:], obuf[:, half, r, tsl],
                                    wons[n][:, r, :],
                                    start=(r == 0), stop=(r == 7))
                        for n in range(2):
                            nsl = slice(n * 512, (n + 1) * 512)
                            nc.vector.tensor_tensor(out=x_sb[:, tt, nsl],
                                                    in0=pw[n][:],
                                                    in1=x_sb[:, tt, nsl],
                                                    op=ALU.add)

              for h in range(4, 8):
                  sinit(h)
                  corr(h)
              wo_round(1)
              for h in range(4):
                  sinit(h)
                  corr(h)
              wo_round(0)

            # ============ LN2 + FFN (x_sb now holds x2) ============
            with tc.tile_pool(name="ffn", bufs=2) as FP, \
                 tc.tile_pool(name="yap", bufs=1) as YA:
                h8 = YA.tile([128, KO, SEG], F8, name="h8")     # 16*h2^T fp8
                ln_phase("2", h8)
                y_acc = YA.tile([128, PT, D], F32, name="yacc")
                for g in range(4):
                    w1g = FP.tile([128, 8, KO, 128], F8, tag="w1g",
                                  name="w1g")
                    nc.sync.dma_start(w1g[:], w1_d.ap()[g])
                    w2g = FP.tile([128, 8, 1024], F8, tag="w2g", name="w2g")
                    nc.sync.dma_start(w2g[:], w2_d.ap()[g])
                    fT = FP.tile([128, 8, 2, 512], F8, tag="fT", name="fT")
                    for kt in range(8):
                      with ns(f"ffn{g}_f{kt}"):
                        ktr = g * 8 + kt
                        pf = [psq(), psq()]
                        for k2 in range(KO // 2):
                            k2s = slice(2 * k2, 2 * k2 + 2)
                            for th in range(2):  # stationary reused
                                nc.tensor.matmul(
                                    pf[th][:], w1g[:, kt, k2s, :],
                                    h8[:, k2s, th * 512:(th + 1) * 512],
                                    start=(k2 == 0),
                                    stop=(k2 == KO // 2 - 1),
                                    perf_mode=PM.DoubleRow)
                        for th in range(2):
                            nc.scalar.activation(fT[:, kt, th, :], pf[th][:],
                                                 AF.Gelu,
                                                 bias=b1T[:, ktr:ktr + 1],
                                                 scale=1.0 / 4096.0)
                    for th in range(2):
                        for tt4 in range(4):
                            tt = th * 4 + tt4
                            t4 = slice(tt4 * 128, (tt4 + 1) * 128)
                            for n in range(2):
                              with ns(f"ffn{g}_{th}_y{tt4}_{n}"):
                                nsl = slice(n * 512, (n + 1) * 512)
                                pg = ps2()
                                for k2 in range(4):
                                    k2s = slice(2 * k2, 2 * k2 + 2)
                                    nc.tensor.matmul(
                                        pg[:], fT[:, k2s, th, t4],
                                        w2g[:, k2s, nsl],
                                        start=(k2 == 0), stop=(k2 == 3),
                                        perf_mode=PM.DoubleRow)
                                if g == 0:
                                    nc.vector.tensor_tensor(
                                        out=y_acc[:, tt, nsl], in0=pg[:],
                                        in1=x_sb[:, tt, nsl], op=ALU.add)
                                elif g < 3:
                                    nc.vector.tensor_tensor(
                                        out=y_acc[:, tt, nsl], in0=pg[:],
                                        in1=y_acc[:, tt, nsl], op=ALU.add)
                                else:
                                    yt = FP.tile([128, 512], BF, tag="yt",
                                                 name="yt")
                                    nc.vector.tensor_tensor(
                                        out=yt[:], in0=pg[:],
                                        in1=y_acc[:, tt, nsl], op=ALU.add)
                                    nc.sync.dma_start(yr[tt][:, nsl], yt[:])

    nc.compile()
    return nc


def _host_prep(inputs):
    x = np.asarray(inputs["x"], np.float32)
    ln1_w = np.asarray(inputs["ln1_w"], np.float32)
    ln1_b = np.asarray(inputs["ln1_b"], np.float32)
    Wq = np.asarray(inputs["Wq"], np.float32)
    Wk = np.asarray(inputs["Wk"], np.float32)
    Wv = np.asarray(inputs["Wv"], np.float32)
    Wo = np.asarray(inputs["Wo"], np.float32)
    ln2_w = np.asarray(inputs["ln2_w"], np.float32)
    ln2_b = np.asarray(inputs["ln2_b"], np.float32)
    W1 = np.asarray(inputs["W1"], np.float32)
    b1 = np.asarray(inputs["b1"], np.float32)
    W2 = np.asarray(inputs["W2"], np.float32)
    b2 = np.asarray(inputs["b2"], np.float32)

    assert np.all(ln1_b == 0) and np.all(ln2_b == 0) and np.all(b2 == 0), \
        "kernel build assumes zero ln1_b/ln2_b/b2 (gated paths not emitted)"

    sc = 1.0 / np.sqrt(np.float64(DK))
    wq_e = ln1_w[:, None] * Wq * sc     # fold 1/sqrt(dk) into Wq
    wk_e = ln1_w[:, None] * Wk
    wv_e = ln1_w[:, None] * Wv
    w1_e = ln2_w[:, None] * W1

    def bf(a):
        return np.ascontiguousarray(a).astype(BF_NP)

    def f8(a):
        return np.ascontiguousarray(np.clip(a, -240.0, 240.0)).astype(F8_NP)

    wqh = wq_e.reshape(KO, 128, H, 128).transpose(2, 1, 0, 3)
    wkh = wk_e.reshape(KO, 128, H, 128).transpose(2, 1, 0, 3)
    wvh = wv_e.reshape(KO, 128, H, 256).transpose(2, 1, 0, 3)
    # per head-pair: [qa | ka | qb | kb | va | vb] -> [4, 128, KO, 1024]
    wqkv = bf(np.concatenate(
        [np.stack([wqh[0::2], wkh[0::2], wqh[1::2], wkh[1::2]], axis=0)
         .transpose(1, 2, 3, 0, 4).reshape(4, 128, KO, 512),
         np.concatenate([wvh[0::2], wvh[1::2]], axis=-1)], axis=-1))

    wo2 = bf(Wo.reshape(2, 8, 128, 2, 512).transpose(0, 2, 3, 1, 4))
    w1t = f8((w1_e * 256.0).reshape(KO, 128, FT, 128).transpose(2, 1, 0, 3)
             .reshape(4, 8, 128, KO, 128).transpose(0, 2, 1, 3, 4))
    w2t = f8(W2.reshape(4, 8, 128, 1024).transpose(0, 2, 1, 3))

    t_ = np.arange(C, dtype=np.float64)
    maskT2 = np.zeros((128, H, 512), np.float64)
    dqbf = np.zeros((128, H, C), np.float64)
    dkcol2 = np.zeros((128, H, NCH), np.float32)
    for h in range(H):
        diff = t_[None, :] - t_[:, None]      # [s, t] -> t - s
        m = np.where(diff >= 0, np.exp(_logb[h] * diff), 0.0)
        for r4 in range(4):
            maskT2[:, h, r4 * 128:(r4 + 1) * 128] = m
        dqbf[:, h, :] = np.exp(_logb[h] * (t_ + 1.0))[None, :]
        dc = np.exp(_logb[h] * C)
        for i in range(NCH):
            dkcol2[:, h, i] = (np.exp(_logb[h] * (C - 1.0 - t_))
                               * dc ** (-(i + 1.0)))
    maskT2 = bf(maskT2)
    dqbf = bf(dqbf)

    b1_e = b1 + ln2_b @ W1
    b1T = np.ascontiguousarray(b1_e.reshape(FT, 128).T).astype(np.float32)

    shared = dict(wqkv=wqkv, wo2=wo2, w1t=w1t, w2t=w2t,
                  maskT2=maskT2, dqbf=dqbf, dkcol2=dkcol2, b1T=b1T)

    in_maps = []
    for c in range(NCORES):
        b, s = c // 4, c % 4
        coefT = np.zeros((128, H, 4), np.float32)
        for h in range(H):
            for j in range(4):
                if j < s:
                    coefT[:, h, j] = np.exp(_logb[h] * (SEG * (s - 1 - j)))
        m = dict(shared)
        m["x"] = bf(x[b, s * SEG:(s + 1) * SEG, :])
        m["coefT"] = coefT
        in_maps.append(m)
    return in_maps


def kernel(**inputs):
    if "nc" not in _NC_CACHE:
        _NC_CACHE["nc"] = _build()
    nc = _NC_CACHE["nc"]
    in_maps = _host_prep(inputs)
    res = run_bass_kernel_spmd(nc, in_maps, core_ids=list(range(NCORES)))
    _NC_CACHE["last_res"] = res
    out = np.zeros((B, L, D), np.float32)
    for c in range(NCORES):
        b, s = c // 4, c % 4
        out[b, s * SEG:(s + 1) * SEG, :] = res.results[c]["y"]
    return out

